# revision 1
# baseline (speedup 1.0000x reference)
"""DeformConv3D (3x3x3, pad 1, stride 1) on 8 Trainium2 NeuronCores.

Sharding: data-parallel over (batch, output-d-slab): core = b*4 + dq handles
batch b, output d-planes [2*dq, 2*dq+2), i.e. 6272 output voxels.

Device pipeline per core (fp16 compute, fp32 accumulate):
  - x is staged in HBM as a dual-parity "w-pair union": channels-last rows of
    128 fp16 values = 2 adjacent w-positions x 64 channels (256B), once for
    even-aligned pairs and once odd-aligned, so ANY (d,h,floor(w)) corner pair
    is one 256B dma_gather element.
  - per (tap k, dh-corner m): batched dma_gather (SWDGE) of 256B elements
  - DVE: multiply gathered pairs by trilinear corner weights, accumulate cols
  - PE: transpose cols to contraction-major, then f16 GEMM over (c, k) with
    PSUM fp32 accumulation
Host side only shards/permutes layouts, computes gather indices/interp
weights from `offset`, and reassembles the output.
"""
import numpy as np
from contextlib import ExitStack

import concourse.bacc as bacc
import concourse.mybir as mybir
import concourse.tile as tile
from concourse import library_config
from concourse.masks import make_identity
from concourse.bass_utils import run_bass_kernel_spmd

F16, F32, I16 = mybir.dt.float16, mybir.dt.float32, mybir.dt.int16

B, C, D, H, W = 2, 64, 8, 56, 56
Cout, K = 64, 27
N_CORES = 8
DQ = 4
DO_SLAB = D // DQ              # 2
P_CORE = DO_SLAB * H * W       # 6272
NPOS = D * H * W               # 25088
NPAIR = NPOS // 2 + 1          # 12545
NU = 2 * NPAIR                 # 25090
JH = [25, 24]
HALF_N = [25 * 128, 24 * 128]
HALF_OFF = [0, 25 * 128]
NKP = 14


def _chunks_of(n):
    out, c0 = [], 0
    while c0 < n:
        cs = min(512, n - c0)
        out.append((c0, cs))
        c0 += cs
    return out


def _build_kernel(nc, out, xsrc, idxA, idxB, wtsA, wtsB, wmat):
    nc.gpsimd.load_library(library_config.mlp)
    with tile.TileContext(nc) as tc, ExitStack() as ctx:
        const = ctx.enter_context(tc.tile_pool(name="const", bufs=1))
        idxp = ctx.enter_context(tc.tile_pool(name="idxp", bufs=4))
        wtp = ctx.enter_context(tc.tile_pool(name="wtp", bufs=3))
        gpool = ctx.enter_context(tc.tile_pool(name="gpool", bufs=3))
        colsp = ctx.enter_context(tc.tile_pool(name="colsp", bufs=3))
        tmpp = ctx.enter_context(tc.tile_pool(name="tmpp", bufs=2))
        rhsp = ctx.enter_context(tc.tile_pool(name="rhsp", bufs=1))
        outp = ctx.enter_context(tc.tile_pool(name="outp", bufs=3))
        psT = ctx.enter_context(tc.tile_pool(name="psT", bufs=4, space="PSUM"))
        psG = ctx.enter_context(tc.tile_pool(name="psG", bufs=2, space="PSUM"))

        ident = const.tile([128, 128], F16)
        make_identity(nc, ident[:])
        wm = const.tile([128, NKP, 64], F16)
        for kp in range(NKP):
            nc.sync.dma_start(wm[:, kp, :], wmat[kp])

        for half in range(2):
            jh = JH[half]
            n = HALF_N[half]
            off = HALF_OFF[half]
            ncols = n // 16
            idx_dram = idxA if half == 0 else idxB
            wts_dram = wtsA if half == 0 else wtsB

            rhs = rhsp.tile([128, NKP, HALF_N[0]], F16, tag="rhs")
            # k=26 leaves rhs[64:, 13] unwritten; zero it so the 0-weight
            # matmul rows can't pull NaNs out of stale SBUF.
            nc.vector.memset(rhs[64:128, NKP - 1, :n], 0.0)

            for k in range(K):
                wt_t = wtp.tile([128, 8 * JH[0]], F16, tag="wt")
                nc.sync.dma_start(wt_t[:, :8 * jh], wts_dram[k])

                cols = colsp.tile([128, jh, 64], F16, tag="cols")
                first = True
                for m in range(4):
                    idx_t = idxp.tile([128, HALF_N[0] // 16], I16, tag="idx")
                    nc.sync.dma_start(idx_t[:, :ncols], idx_dram[k * 4 + m])
                    g = gpool.tile([128, jh, 128], F16, tag="g")
                    nc.gpsimd.dma_gather(
                        g[:], xsrc[:], idx_t[:, :ncols], n, n, 128,
                        single_packet=False,
                    )
                    for h in range(2):
                        wb = wt_t[:, (m * 2 + h) * jh:(m * 2 + h + 1) * jh]
                        wb = wb.to_broadcast([128, jh, 64])
                        gh = g[:, :, h * 64:(h + 1) * 64]
                        if first:
                            nc.vector.tensor_tensor(
                                out=cols[:], in0=gh, in1=wb,
                                op=mybir.AluOpType.mult)
                            first = False
                        else:
                            t = tmpp.tile([128, jh, 64], F16, tag="tmp")
                            nc.vector.tensor_tensor(
                                out=t[:], in0=gh, in1=wb,
                                op=mybir.AluOpType.mult)
                            nc.vector.tensor_tensor(
                                out=cols[:], in0=cols[:], in1=t[:],
                                op=mybir.AluOpType.add)

                kp, s = divmod(k, 2)
                for j in range(jh):
                    pt = psT.tile([64, 128], F16, tag="pt")
                    nc.tensor.transpose(
                        out=pt[:], in_=cols[:, j, :], identity=ident[:])
                    nc.scalar.copy(
                        out=rhs[s * 64:(s + 1) * 64, kp, j * 128:(j + 1) * 128],
                        in_=pt[:])

            for (c0, cs) in _chunks_of(n):
                po = psG.tile([64, 512], F32, tag="po")
                for kp in range(NKP):
                    nc.tensor.matmul(
                        out=po[:, :cs], lhsT=wm[:, kp, :],
                        rhs=rhs[:, kp, c0:c0 + cs],
                        start=(kp == 0), stop=(kp == NKP - 1))
                ob = outp.tile([64, 512], F32, tag="ob")
                nc.vector.tensor_copy(out=ob[:, :cs], in_=po[:, :cs])
                nc.sync.dma_start(out[:, off + c0:off + c0 + cs], ob[:, :cs])


_NC_CACHE = None


def _get_nc():
    global _NC_CACHE
    if _NC_CACHE is None:
        nc = bacc.Bacc("TRN2", target_bir_lowering=False, debug=False,
                       num_devices=N_CORES)
        xsrc = nc.dram_tensor("xsrc", [NU, 2 * C], F16, kind="ExternalInput")
        idxA = nc.dram_tensor("idxA", [K * 4, 128, HALF_N[0] // 16], I16,
                              kind="ExternalInput")
        idxB = nc.dram_tensor("idxB", [K * 4, 128, HALF_N[1] // 16], I16,
                              kind="ExternalInput")
        wtsA = nc.dram_tensor("wtsA", [K, 128, 8 * JH[0]], F16,
                              kind="ExternalInput")
        wtsB = nc.dram_tensor("wtsB", [K, 128, 8 * JH[1]], F16,
                              kind="ExternalInput")
        wmat = nc.dram_tensor("wmat", [NKP, 128, Cout], F16,
                              kind="ExternalInput")
        out = nc.dram_tensor("out", [Cout, P_CORE], F32, kind="ExternalOutput")
        _build_kernel(nc, out[:, :], xsrc[:, :], idxA, idxB, wtsA, wtsB,
                      wmat)
        nc.compile()
        _NC_CACHE = nc
    return _NC_CACHE


# ---------------- host-side prep ----------------

def _build_union(xb):
    x_cl = np.ascontiguousarray(np.asarray(xb).transpose(1, 2, 3, 0))
    x_cl = x_cl.reshape(NPOS, C)
    F = np.zeros((NPOS + 4, C), np.float16)
    F[1:NPOS + 1] = x_cl.astype(np.float16)
    copyA = F[0:2 * NPAIR].reshape(NPAIR, 2 * C)
    copyB = F[1:2 * NPAIR + 1].reshape(NPAIR, 2 * C)
    return np.ascontiguousarray(np.concatenate([copyA, copyB], 0))


def _host_idx_weights(off_core, dq):
    off = np.asarray(off_core).reshape(K, 3, P_CORE).astype(np.float32)
    pidx = np.arange(P_CORE)
    do = (pidx // (H * W)) + dq * DO_SLAB
    ho = (pidx // W) % H
    wo = pidx % W
    kk = np.arange(K)
    kd = (kk // 9).astype(np.float32)
    kh = ((kk // 3) % 3).astype(np.float32)
    kw = (kk % 3).astype(np.float32)

    pd = off[:, 0] + kd[:, None] + (do[None, :] - 1.0)
    ph = off[:, 1] + kh[:, None] + (ho[None, :] - 1.0)
    pw = off[:, 2] + kw[:, None] + (wo[None, :] - 1.0)

    d0 = np.floor(pd); fd = pd - d0
    h0 = np.floor(ph); fh = ph - h0
    w0 = np.floor(pw); fw = pw - w0

    w0c = np.clip(w0, -1, W - 1)
    vw0 = ((w0 >= 0) & (w0 <= W - 1)).astype(np.float32)
    vw1 = ((w0 >= -1) & (w0 <= W - 2)).astype(np.float32)
    ww0 = (1.0 - fw) * vw0
    ww1 = fw * vw1

    idx = np.zeros((K, 4, P_CORE), np.int16)
    wts = np.zeros((K, 4, 2, P_CORE), np.float16)
    for m, (bd, bh) in enumerate([(0, 0), (0, 1), (1, 0), (1, 1)]):
        dc = np.clip(d0 + bd, 0, D - 1)
        hc = np.clip(h0 + bh, 0, H - 1)
        vd = ((d0 + bd >= 0) & (d0 + bd <= D - 1)).astype(np.float32)
        vh = ((h0 + bh >= 0) & (h0 + bh <= H - 1)).astype(np.float32)
        wd = (fd if bd else 1.0 - fd) * vd
        wh = (fh if bh else 1.0 - fh) * vh
        lin = (dc * H + hc) * W + w0c
        i = lin + 1.0
        q = i % 2
        idx[:, m] = ((i - q) / 2 + q * NPAIR).astype(np.int16)
        wts[:, m, 0] = (wd * wh * ww0).astype(np.float16)
        wts[:, m, 1] = (wd * wh * ww1).astype(np.float16)
    return idx, wts


def _wrap_idx_batch(vals, n):
    """vals [..., n] -> wrapped tiles [..., 128, n//16]."""
    lead = vals.shape[:-1]
    w = vals.reshape(*lead, n // 16, 16)
    w = np.swapaxes(w, -1, -2)  # [..., 16, n//16]
    return np.broadcast_to(
        w[..., None, :, :], (*lead, 8, 16, n // 16)
    ).reshape(*lead, 128, n // 16)


def _pack_wmat(weight):
    wk = np.asarray(weight).reshape(Cout, C, K).astype(np.float16)
    lhsT = np.zeros((NKP, 128, Cout), np.float16)
    for kp in range(NKP):
        for s in range(2):
            k = 2 * kp + s
            if k < K:
                lhsT[kp, s * 64:(s + 1) * 64, :] = wk[:, :, k].T
    return lhsT


def _core_inputs(union_b, offset, lhsT, core):
    b, dq = core // DQ, core % DQ
    off_core = np.asarray(offset[b, :, dq * DO_SLAB:(dq + 1) * DO_SLAB])
    idx, wts = _host_idx_weights(off_core, dq)

    idxA = np.ascontiguousarray(
        _wrap_idx_batch(idx[:, :, :HALF_N[0]].reshape(K * 4, HALF_N[0]),
                        HALF_N[0]))
    idxB = np.ascontiguousarray(
        _wrap_idx_batch(idx[:, :, HALF_N[0]:].reshape(K * 4, HALF_N[1]),
                        HALF_N[1]))
    # weights: [K, 4, 2, P] -> per half [K, 128, (m,h,j)]
    wA = wts[:, :, :, :HALF_N[0]].reshape(K, 4, 2, JH[0], 128)
    wA = np.ascontiguousarray(
        wA.transpose(0, 4, 1, 2, 3).reshape(K, 128, 8 * JH[0]))
    wB = wts[:, :, :, HALF_N[0]:].reshape(K, 4, 2, JH[1], 128)
    wB = np.ascontiguousarray(
        wB.transpose(0, 4, 1, 2, 3).reshape(K, 128, 8 * JH[1]))
    return dict(xsrc=union_b, idxA=idxA, idxB=idxB, wtsA=wA, wtsB=wB,
                wmat=lhsT)


def make_in_maps(x, offset, weight):
    lhsT = _pack_wmat(weight)
    unions = [_build_union(np.asarray(x)[b]) for b in range(B)]
    return [
        _core_inputs(unions[core // DQ], np.asarray(offset), lhsT, core)
        for core in range(N_CORES)
    ]


def assemble_output(results):
    out = np.zeros((B, Cout, D, H, W), np.float32)
    for core in range(N_CORES):
        b, dq = core // DQ, core % DQ
        o = results[core]["out"]
        out[b, :, dq * DO_SLAB:(dq + 1) * DO_SLAB] = o.reshape(
            Cout, DO_SLAB, H, W)
    return out


def kernel(x, offset, weight):
    x = np.asarray(x)
    offset = np.asarray(offset)
    weight = np.asarray(weight)
    nc = _get_nc()
    in_maps = make_in_maps(x, offset, weight)
    res = run_bass_kernel_spmd(nc, in_maps, core_ids=list(range(N_CORES)))
    return assemble_output(res.results)


# revision 3
# speedup vs baseline: 165.3969x; 165.3969x over previous
"""DeformConv3D (3x3x3, pad 1, stride 1) on 8 Trainium2 NeuronCores.

Sharding: data-parallel over (batch, output-d-slab): core = b*4 + dq handles
batch b, output d-planes [2*dq, 2*dq+2), i.e. 6272 output voxels.

Device pipeline per core (fp16 compute, fp32 accumulate):
  - x is staged in HBM as a dual-parity "w-pair union": channels-last rows of
    128 fp16 values = 2 adjacent w-positions x 64 channels (256B), once for
    even-aligned pairs and once odd-aligned, so ANY (d,h,floor(w)) corner pair
    is one 256B dma_gather element.
  - per (tap k, dh-corner m): batched dma_gather (SWDGE) of 256B elements
  - DVE: multiply gathered pairs by trilinear corner weights, accumulate cols
  - PE: transpose cols to contraction-major, then f16 GEMM over (c, k) with
    PSUM fp32 accumulation
Host side only shards/permutes layouts, computes gather indices/interp
weights from `offset`, and reassembles the output.
"""
import numpy as np
from contextlib import ExitStack

import concourse.bacc as bacc
import concourse.mybir as mybir
import concourse.tile as tile
from concourse import library_config
from concourse.masks import make_identity
from concourse.bass_utils import run_bass_kernel_spmd

F16, F32, I16 = mybir.dt.float16, mybir.dt.float32, mybir.dt.int16

B, C, D, H, W = 2, 64, 8, 56, 56
Cout, K = 64, 27
N_CORES = 8
DQ = 4
DO_SLAB = D // DQ              # 2
P_CORE = DO_SLAB * H * W       # 6272
NPOS = D * H * W               # 25088
NPAIR = NPOS // 2 + 1          # 12545
NU = 2 * NPAIR                 # 25090
JH = [25, 24]
HALF_N = [25 * 128, 24 * 128]
HALF_OFF = [0, 25 * 128]
NKP = 14


def _chunks_of(n):
    out, c0 = [], 0
    while c0 < n:
        cs = min(512, n - c0)
        out.append((c0, cs))
        c0 += cs
    return out


def _build_kernel(nc, out, xsrc, idxA, idxB, wtsA, wtsB, wmat):
    nc.gpsimd.load_library(library_config.mlp)
    with tile.TileContext(nc) as tc, ExitStack() as ctx:
        const = ctx.enter_context(tc.tile_pool(name="const", bufs=1))
        idxp = ctx.enter_context(tc.tile_pool(name="idxp", bufs=4))
        wtp = ctx.enter_context(tc.tile_pool(name="wtp", bufs=3))
        gpool = ctx.enter_context(tc.tile_pool(name="gpool", bufs=3))
        colsp = ctx.enter_context(tc.tile_pool(name="colsp", bufs=3))
        tmpp = ctx.enter_context(tc.tile_pool(name="tmpp", bufs=2))
        rhsp = ctx.enter_context(tc.tile_pool(name="rhsp", bufs=1))
        outp = ctx.enter_context(tc.tile_pool(name="outp", bufs=3))
        psT = ctx.enter_context(tc.tile_pool(name="psT", bufs=4, space="PSUM"))
        psG = ctx.enter_context(tc.tile_pool(name="psG", bufs=2, space="PSUM"))

        ident = const.tile([128, 128], F16)
        make_identity(nc, ident[:])
        wm = const.tile([128, NKP, 64], F16)
        for kp in range(NKP):
            nc.scalar.dma_start(wm[:, kp, :], wmat[kp])

        for half in range(2):
            jh = JH[half]
            n = HALF_N[half]
            off = HALF_OFF[half]
            ncols = n // 16
            idx_dram = idxA if half == 0 else idxB
            wts_dram = wtsA if half == 0 else wtsB

            rhs = rhsp.tile([128, NKP, HALF_N[0]], F16, tag="rhs")
            # k=26 leaves rhs[64:, 13] unwritten; zero it so the 0-weight
            # matmul rows can't pull NaNs out of stale SBUF.
            nc.vector.memset(rhs[64:128, NKP - 1, :n], 0.0)

            for k in range(K):
                wt_t = wtp.tile([128, 8 * JH[0]], F16, tag="wt")
                nc.scalar.dma_start(wt_t[:, :8 * jh], wts_dram[k])

                idx_t = idxp.tile([128, 4, HALF_N[0] // 16], I16, tag="idx")
                nc.sync.dma_start(
                    idx_t[:, :, :ncols],
                    idx_dram[k].rearrange("p (m c) -> p m c", m=4))

                cols = colsp.tile([128, jh, 64], F16, tag="cols")
                first = True
                for m in range(4):
                    g = gpool.tile([128, jh, 128], F16, tag="g")
                    nc.gpsimd.dma_gather(
                        g[:], xsrc[:], idx_t[:, m, :ncols], n, n, 128,
                        single_packet=False,
                    )
                    for h in range(2):
                        wb = wt_t[:, (m * 2 + h) * jh:(m * 2 + h + 1) * jh]
                        wb = wb.to_broadcast([128, jh, 64])
                        gh = g[:, :, h * 64:(h + 1) * 64]
                        if first:
                            nc.vector.tensor_tensor(
                                out=cols[:], in0=gh, in1=wb,
                                op=mybir.AluOpType.mult)
                            first = False
                        else:
                            t = tmpp.tile([128, jh, 64], F16, tag="tmp")
                            nc.vector.tensor_tensor(
                                out=t[:], in0=gh, in1=wb,
                                op=mybir.AluOpType.mult)
                            nc.vector.tensor_tensor(
                                out=cols[:], in0=cols[:], in1=t[:],
                                op=mybir.AluOpType.add)

                kp, s = divmod(k, 2)
                for j in range(jh):
                    pt = psT.tile([64, 128], F16, tag="pt")
                    nc.tensor.transpose(
                        out=pt[:], in_=cols[:, j, :], identity=ident[:])
                    nc.scalar.copy(
                        out=rhs[s * 64:(s + 1) * 64, kp, j * 128:(j + 1) * 128],
                        in_=pt[:])

            for (c0, cs) in _chunks_of(n):
                po = psG.tile([64, 512], F32, tag="po")
                for kp in range(NKP):
                    nc.tensor.matmul(
                        out=po[:, :cs], lhsT=wm[:, kp, :],
                        rhs=rhs[:, kp, c0:c0 + cs],
                        start=(kp == 0), stop=(kp == NKP - 1))
                ob = outp.tile([64, 512], F32, tag="ob")
                nc.vector.tensor_copy(out=ob[:, :cs], in_=po[:, :cs])
                nc.sync.dma_start(out[:, off + c0:off + c0 + cs], ob[:, :cs])


_NC_CACHE = None


def _get_nc():
    global _NC_CACHE
    if _NC_CACHE is None:
        nc = bacc.Bacc("TRN2", target_bir_lowering=False, debug=False,
                       num_devices=N_CORES)
        xsrc = nc.dram_tensor("xsrc", [NU, 2 * C], F16, kind="ExternalInput")
        idxA = nc.dram_tensor("idxA", [K, 128, 4 * (HALF_N[0] // 16)], I16,
                              kind="ExternalInput")
        idxB = nc.dram_tensor("idxB", [K, 128, 4 * (HALF_N[1] // 16)], I16,
                              kind="ExternalInput")
        wtsA = nc.dram_tensor("wtsA", [K, 128, 8 * JH[0]], F16,
                              kind="ExternalInput")
        wtsB = nc.dram_tensor("wtsB", [K, 128, 8 * JH[1]], F16,
                              kind="ExternalInput")
        wmat = nc.dram_tensor("wmat", [NKP, 128, Cout], F16,
                              kind="ExternalInput")
        out = nc.dram_tensor("out", [Cout, P_CORE], F32, kind="ExternalOutput")
        _build_kernel(nc, out[:, :], xsrc[:, :], idxA, idxB, wtsA, wtsB,
                      wmat)
        nc.compile()
        _NC_CACHE = nc
    return _NC_CACHE


# ---------------- host-side prep ----------------

def _build_union(xb):
    x_cl = np.ascontiguousarray(np.asarray(xb).transpose(1, 2, 3, 0))
    x_cl = x_cl.reshape(NPOS, C)
    F = np.zeros((NPOS + 4, C), np.float16)
    F[1:NPOS + 1] = x_cl.astype(np.float16)
    copyA = F[0:2 * NPAIR].reshape(NPAIR, 2 * C)
    copyB = F[1:2 * NPAIR + 1].reshape(NPAIR, 2 * C)
    return np.ascontiguousarray(np.concatenate([copyA, copyB], 0))


def _host_idx_weights(off_core, dq):
    off = np.asarray(off_core).reshape(K, 3, P_CORE).astype(np.float32)
    pidx = np.arange(P_CORE)
    do = (pidx // (H * W)) + dq * DO_SLAB
    ho = (pidx // W) % H
    wo = pidx % W
    kk = np.arange(K)
    kd = (kk // 9).astype(np.float32)
    kh = ((kk // 3) % 3).astype(np.float32)
    kw = (kk % 3).astype(np.float32)

    pd = off[:, 0] + kd[:, None] + (do[None, :] - 1.0)
    ph = off[:, 1] + kh[:, None] + (ho[None, :] - 1.0)
    pw = off[:, 2] + kw[:, None] + (wo[None, :] - 1.0)

    d0 = np.floor(pd); fd = pd - d0
    h0 = np.floor(ph); fh = ph - h0
    w0 = np.floor(pw); fw = pw - w0

    w0c = np.clip(w0, -1, W - 1)
    vw0 = ((w0 >= 0) & (w0 <= W - 1)).astype(np.float32)
    vw1 = ((w0 >= -1) & (w0 <= W - 2)).astype(np.float32)
    ww0 = (1.0 - fw) * vw0
    ww1 = fw * vw1

    idx = np.zeros((K, 4, P_CORE), np.int16)
    wts = np.zeros((K, 4, 2, P_CORE), np.float16)
    for m, (bd, bh) in enumerate([(0, 0), (0, 1), (1, 0), (1, 1)]):
        dc = np.clip(d0 + bd, 0, D - 1)
        hc = np.clip(h0 + bh, 0, H - 1)
        vd = ((d0 + bd >= 0) & (d0 + bd <= D - 1)).astype(np.float32)
        vh = ((h0 + bh >= 0) & (h0 + bh <= H - 1)).astype(np.float32)
        wd = (fd if bd else 1.0 - fd) * vd
        wh = (fh if bh else 1.0 - fh) * vh
        lin = (dc * H + hc) * W + w0c
        i = lin + 1.0
        q = i % 2
        idx[:, m] = ((i - q) / 2 + q * NPAIR).astype(np.int16)
        wts[:, m, 0] = (wd * wh * ww0).astype(np.float16)
        wts[:, m, 1] = (wd * wh * ww1).astype(np.float16)
    return idx, wts


def _wrap_idx_batch(vals, n):
    """vals [..., n] -> wrapped tiles [..., 128, n//16]."""
    lead = vals.shape[:-1]
    w = vals.reshape(*lead, n // 16, 16)
    w = np.swapaxes(w, -1, -2)  # [..., 16, n//16]
    return np.broadcast_to(
        w[..., None, :, :], (*lead, 8, 16, n // 16)
    ).reshape(*lead, 128, n // 16)


def _pack_wmat(weight):
    wk = np.asarray(weight).reshape(Cout, C, K).astype(np.float16)
    lhsT = np.zeros((NKP, 128, Cout), np.float16)
    for kp in range(NKP):
        for s in range(2):
            k = 2 * kp + s
            if k < K:
                lhsT[kp, s * 64:(s + 1) * 64, :] = wk[:, :, k].T
    return lhsT


def _core_inputs(union_b, offset, lhsT, core):
    b, dq = core // DQ, core % DQ
    off_core = np.asarray(offset[b, :, dq * DO_SLAB:(dq + 1) * DO_SLAB])
    idx, wts = _host_idx_weights(off_core, dq)

    idxA = _wrap_idx_batch(idx[:, :, :HALF_N[0]].reshape(K * 4, HALF_N[0]),
                           HALF_N[0]).reshape(K, 4, 128, HALF_N[0] // 16)
    idxA = np.ascontiguousarray(
        idxA.transpose(0, 2, 1, 3).reshape(K, 128, 4 * (HALF_N[0] // 16)))
    idxB = _wrap_idx_batch(idx[:, :, HALF_N[0]:].reshape(K * 4, HALF_N[1]),
                           HALF_N[1]).reshape(K, 4, 128, HALF_N[1] // 16)
    idxB = np.ascontiguousarray(
        idxB.transpose(0, 2, 1, 3).reshape(K, 128, 4 * (HALF_N[1] // 16)))
    # weights: [K, 4, 2, P] -> per half [K, 128, (m,h,j)]
    wA = wts[:, :, :, :HALF_N[0]].reshape(K, 4, 2, JH[0], 128)
    wA = np.ascontiguousarray(
        wA.transpose(0, 4, 1, 2, 3).reshape(K, 128, 8 * JH[0]))
    wB = wts[:, :, :, HALF_N[0]:].reshape(K, 4, 2, JH[1], 128)
    wB = np.ascontiguousarray(
        wB.transpose(0, 4, 1, 2, 3).reshape(K, 128, 8 * JH[1]))
    return dict(xsrc=union_b, idxA=idxA, idxB=idxB, wtsA=wA, wtsB=wB,
                wmat=lhsT)


def make_in_maps(x, offset, weight):
    lhsT = _pack_wmat(weight)
    unions = [_build_union(np.asarray(x)[b]) for b in range(B)]
    return [
        _core_inputs(unions[core // DQ], np.asarray(offset), lhsT, core)
        for core in range(N_CORES)
    ]


def assemble_output(results):
    out = np.zeros((B, Cout, D, H, W), np.float32)
    for core in range(N_CORES):
        b, dq = core // DQ, core % DQ
        o = results[core]["out"]
        out[b, :, dq * DO_SLAB:(dq + 1) * DO_SLAB] = o.reshape(
            Cout, DO_SLAB, H, W)
    return out


def kernel(x, offset, weight):
    x = np.asarray(x)
    offset = np.asarray(offset)
    weight = np.asarray(weight)
    nc = _get_nc()
    in_maps = make_in_maps(x, offset, weight)
    res = run_bass_kernel_spmd(nc, in_maps, core_ids=list(range(N_CORES)))
    return assemble_output(res.results)


# revision 4
# speedup vs baseline: 174.9360x; 1.0577x over previous
"""DeformConv3D (3x3x3, pad 1, stride 1) on 8 Trainium2 NeuronCores.

Sharding: data-parallel over (batch, output-d-slab): core = b*4 + dq handles
batch b, output d-planes [2*dq, 2*dq+2), i.e. 6272 output voxels.

Device pipeline per core (fp16 compute, fp32 accumulate):
  - x is staged in HBM as a dual-parity "w-pair union": channels-last rows of
    128 fp16 values = 2 adjacent w-positions x 64 channels (256B), once for
    even-aligned pairs and once odd-aligned, so ANY (d,h,floor(w)) corner pair
    is one 256B dma_gather element.
  - per (tap k, dh-corner m): batched dma_gather (SWDGE) of 256B elements
  - DVE: multiply gathered pairs by trilinear corner weights, accumulate cols
  - PE: transpose cols to contraction-major, then f16 GEMM over (c, k) with
    PSUM fp32 accumulation
Host side only shards/permutes layouts, computes gather indices/interp
weights from `offset`, and reassembles the output.
"""
import os
import numpy as np
from contextlib import ExitStack

import concourse.bacc as bacc
import concourse.mybir as mybir
import concourse.tile as tile
from concourse import library_config
from concourse.masks import make_identity
from concourse.bass_utils import run_bass_kernel_spmd

F16, F32, I16 = mybir.dt.float16, mybir.dt.float32, mybir.dt.int16
_ABLATE = os.environ.get("DEFORM_ABLATE", "")
_NQUEUES = int(os.environ.get("DEFORM_NQUEUES", "1"))

B, C, D, H, W = 2, 64, 8, 56, 56
Cout, K = 64, 27
N_CORES = 8
DQ = 4
DO_SLAB = D // DQ              # 2
P_CORE = DO_SLAB * H * W       # 6272
NPOS = D * H * W               # 25088
NPAIR = NPOS // 2 + 1          # 12545
NU = 2 * NPAIR                 # 25090
JH = [25, 24]
HALF_N = [25 * 128, 24 * 128]
HALF_OFF = [0, 25 * 128]
NKP = 14


def _chunks_of(n):
    out, c0 = [], 0
    while c0 < n:
        cs = min(512, n - c0)
        out.append((c0, cs))
        c0 += cs
    return out


def _build_kernel(nc, out, xsrc, idxA, idxB, wtsA, wtsB, wmat):
    nc.gpsimd.load_library(library_config.mlp)
    with tile.TileContext(nc) as tc, ExitStack() as ctx:
        const = ctx.enter_context(tc.tile_pool(name="const", bufs=1))
        idxp = ctx.enter_context(tc.tile_pool(name="idxp", bufs=4))
        wtp = ctx.enter_context(tc.tile_pool(name="wtp", bufs=3))
        gpool = ctx.enter_context(tc.tile_pool(name="gpool", bufs=3))
        colsp = ctx.enter_context(tc.tile_pool(name="colsp", bufs=3))
        tmpp = ctx.enter_context(tc.tile_pool(name="tmpp", bufs=2))
        rhsp = ctx.enter_context(tc.tile_pool(name="rhsp", bufs=1))
        outp = ctx.enter_context(tc.tile_pool(name="outp", bufs=3))
        psT = ctx.enter_context(tc.tile_pool(name="psT", bufs=4, space="PSUM"))
        psG = ctx.enter_context(tc.tile_pool(name="psG", bufs=2, space="PSUM"))

        ident = const.tile([128, 128], F16)
        make_identity(nc, ident[:])
        wm = const.tile([128, NKP, 64], F16)
        for kp in range(NKP):
            nc.scalar.dma_start(wm[:, kp, :], wmat[kp])

        for half in range(2):
            jh = JH[half]
            n = HALF_N[half]
            off = HALF_OFF[half]
            ncols = n // 16
            idx_dram = idxA if half == 0 else idxB
            wts_dram = wtsA if half == 0 else wtsB

            rhs = rhsp.tile([128, NKP, HALF_N[0]], F16, tag="rhs")
            # k=26 leaves rhs[64:, 13] unwritten; zero it so the 0-weight
            # matmul rows can't pull NaNs out of stale SBUF.
            nc.vector.memset(rhs[64:128, NKP - 1, :n], 0.0)

            for k in range(K):
                wt_t = wtp.tile([128, 8 * JH[0]], F16, tag="wt")
                nc.scalar.dma_start(wt_t[:, :8 * jh], wts_dram[k])

                idx_t = idxp.tile([128, 4, HALF_N[0] // 16], I16, tag="idx")
                nc.sync.dma_start(
                    idx_t[:, :, :ncols],
                    idx_dram[k].rearrange("p (m c) -> p m c", m=4))

                cols = colsp.tile([128, jh, 64], F16, tag="cols")
                first = True
                for m in range(4):
                    g = gpool.tile([128, jh, 128], F16, tag="g")
                    if _ABLATE != "nogather":
                        nc.gpsimd.dma_gather(
                            g[:], xsrc[:], idx_t[:, m, :ncols], n, n, 128,
                            single_packet=False,
                            queue_num=(k * 4 + m) % _NQUEUES,
                        )
                    if _ABLATE == "gatheronly":
                        continue
                    for h in range(2):
                        if _ABLATE == "gatheronly":
                            break
                        wb = wt_t[:, (m * 2 + h) * jh:(m * 2 + h + 1) * jh]
                        wb = wb.to_broadcast([128, jh, 64])
                        gh = g[:, :, h * 64:(h + 1) * 64]
                        if first:
                            nc.vector.tensor_tensor(
                                out=cols[:], in0=gh, in1=wb,
                                op=mybir.AluOpType.mult)
                            first = False
                        else:
                            t = tmpp.tile([128, jh, 64], F16, tag="tmp")
                            nc.vector.tensor_tensor(
                                out=t[:], in0=gh, in1=wb,
                                op=mybir.AluOpType.mult)
                            nc.vector.tensor_tensor(
                                out=cols[:], in0=cols[:], in1=t[:],
                                op=mybir.AluOpType.add)

                kp, s = divmod(k, 2)
                if _ABLATE == "gatheronly":
                    continue
                for j in range(jh):
                    pt = psT.tile([64, 128], F16, tag="pt")
                    nc.tensor.transpose(
                        out=pt[:], in_=cols[:, j, :], identity=ident[:])
                    nc.scalar.copy(
                        out=rhs[s * 64:(s + 1) * 64, kp, j * 128:(j + 1) * 128],
                        in_=pt[:])

            for (c0, cs) in _chunks_of(n):
                po = psG.tile([64, 512], F32, tag="po")
                for kp in range(NKP):
                    nc.tensor.matmul(
                        out=po[:, :cs], lhsT=wm[:, kp, :],
                        rhs=rhs[:, kp, c0:c0 + cs],
                        start=(kp == 0), stop=(kp == NKP - 1))
                ob = outp.tile([64, 512], F32, tag="ob")
                nc.vector.tensor_copy(out=ob[:, :cs], in_=po[:, :cs])
                nc.sync.dma_start(out[:, off + c0:off + c0 + cs], ob[:, :cs])


_NC_CACHE = None


def _get_nc():
    global _NC_CACHE
    if _NC_CACHE is None:
        nc = bacc.Bacc("TRN2", target_bir_lowering=False, debug=False,
                       num_devices=N_CORES, num_swdge_queues=_NQUEUES)
        xsrc = nc.dram_tensor("xsrc", [NU, 2 * C], F16, kind="ExternalInput")
        idxA = nc.dram_tensor("idxA", [K, 128, 4 * (HALF_N[0] // 16)], I16,
                              kind="ExternalInput")
        idxB = nc.dram_tensor("idxB", [K, 128, 4 * (HALF_N[1] // 16)], I16,
                              kind="ExternalInput")
        wtsA = nc.dram_tensor("wtsA", [K, 128, 8 * JH[0]], F16,
                              kind="ExternalInput")
        wtsB = nc.dram_tensor("wtsB", [K, 128, 8 * JH[1]], F16,
                              kind="ExternalInput")
        wmat = nc.dram_tensor("wmat", [NKP, 128, Cout], F16,
                              kind="ExternalInput")
        out = nc.dram_tensor("out", [Cout, P_CORE], F32, kind="ExternalOutput")
        _build_kernel(nc, out[:, :], xsrc[:, :], idxA, idxB, wtsA, wtsB,
                      wmat)
        nc.compile()
        _NC_CACHE = nc
    return _NC_CACHE


# ---------------- host-side prep ----------------

def _build_union(xb):
    x_cl = np.ascontiguousarray(np.asarray(xb).transpose(1, 2, 3, 0))
    x_cl = x_cl.reshape(NPOS, C)
    F = np.zeros((NPOS + 4, C), np.float16)
    F[1:NPOS + 1] = x_cl.astype(np.float16)
    copyA = F[0:2 * NPAIR].reshape(NPAIR, 2 * C)
    copyB = F[1:2 * NPAIR + 1].reshape(NPAIR, 2 * C)
    return np.ascontiguousarray(np.concatenate([copyA, copyB], 0))


def _host_idx_weights(off_core, dq):
    off = np.asarray(off_core).reshape(K, 3, P_CORE).astype(np.float32)
    pidx = np.arange(P_CORE)
    do = (pidx // (H * W)) + dq * DO_SLAB
    ho = (pidx // W) % H
    wo = pidx % W
    kk = np.arange(K)
    kd = (kk // 9).astype(np.float32)
    kh = ((kk // 3) % 3).astype(np.float32)
    kw = (kk % 3).astype(np.float32)

    pd = off[:, 0] + kd[:, None] + (do[None, :] - 1.0)
    ph = off[:, 1] + kh[:, None] + (ho[None, :] - 1.0)
    pw = off[:, 2] + kw[:, None] + (wo[None, :] - 1.0)

    d0 = np.floor(pd); fd = pd - d0
    h0 = np.floor(ph); fh = ph - h0
    w0 = np.floor(pw); fw = pw - w0

    w0c = np.clip(w0, -1, W - 1)
    vw0 = ((w0 >= 0) & (w0 <= W - 1)).astype(np.float32)
    vw1 = ((w0 >= -1) & (w0 <= W - 2)).astype(np.float32)
    ww0 = (1.0 - fw) * vw0
    ww1 = fw * vw1

    idx = np.zeros((K, 4, P_CORE), np.int16)
    wts = np.zeros((K, 4, 2, P_CORE), np.float16)
    for m, (bd, bh) in enumerate([(0, 0), (0, 1), (1, 0), (1, 1)]):
        dc = np.clip(d0 + bd, 0, D - 1)
        hc = np.clip(h0 + bh, 0, H - 1)
        vd = ((d0 + bd >= 0) & (d0 + bd <= D - 1)).astype(np.float32)
        vh = ((h0 + bh >= 0) & (h0 + bh <= H - 1)).astype(np.float32)
        wd = (fd if bd else 1.0 - fd) * vd
        wh = (fh if bh else 1.0 - fh) * vh
        lin = (dc * H + hc) * W + w0c
        i = lin + 1.0
        q = i % 2
        idx[:, m] = ((i - q) / 2 + q * NPAIR).astype(np.int16)
        wts[:, m, 0] = (wd * wh * ww0).astype(np.float16)
        wts[:, m, 1] = (wd * wh * ww1).astype(np.float16)
    return idx, wts


def _wrap_idx_batch(vals, n):
    """vals [..., n] -> wrapped tiles [..., 128, n//16]."""
    lead = vals.shape[:-1]
    w = vals.reshape(*lead, n // 16, 16)
    w = np.swapaxes(w, -1, -2)  # [..., 16, n//16]
    return np.broadcast_to(
        w[..., None, :, :], (*lead, 8, 16, n // 16)
    ).reshape(*lead, 128, n // 16)


def _pack_wmat(weight):
    wk = np.asarray(weight).reshape(Cout, C, K).astype(np.float16)
    lhsT = np.zeros((NKP, 128, Cout), np.float16)
    for kp in range(NKP):
        for s in range(2):
            k = 2 * kp + s
            if k < K:
                lhsT[kp, s * 64:(s + 1) * 64, :] = wk[:, :, k].T
    return lhsT


def _core_inputs(union_b, offset, lhsT, core):
    b, dq = core // DQ, core % DQ
    off_core = np.asarray(offset[b, :, dq * DO_SLAB:(dq + 1) * DO_SLAB])
    idx, wts = _host_idx_weights(off_core, dq)

    idxA = _wrap_idx_batch(idx[:, :, :HALF_N[0]].reshape(K * 4, HALF_N[0]),
                           HALF_N[0]).reshape(K, 4, 128, HALF_N[0] // 16)
    idxA = np.ascontiguousarray(
        idxA.transpose(0, 2, 1, 3).reshape(K, 128, 4 * (HALF_N[0] // 16)))
    idxB = _wrap_idx_batch(idx[:, :, HALF_N[0]:].reshape(K * 4, HALF_N[1]),
                           HALF_N[1]).reshape(K, 4, 128, HALF_N[1] // 16)
    idxB = np.ascontiguousarray(
        idxB.transpose(0, 2, 1, 3).reshape(K, 128, 4 * (HALF_N[1] // 16)))
    # weights: [K, 4, 2, P] -> per half [K, 128, (m,h,j)]
    wA = wts[:, :, :, :HALF_N[0]].reshape(K, 4, 2, JH[0], 128)
    wA = np.ascontiguousarray(
        wA.transpose(0, 4, 1, 2, 3).reshape(K, 128, 8 * JH[0]))
    wB = wts[:, :, :, HALF_N[0]:].reshape(K, 4, 2, JH[1], 128)
    wB = np.ascontiguousarray(
        wB.transpose(0, 4, 1, 2, 3).reshape(K, 128, 8 * JH[1]))
    return dict(xsrc=union_b, idxA=idxA, idxB=idxB, wtsA=wA, wtsB=wB,
                wmat=lhsT)


def make_in_maps(x, offset, weight):
    lhsT = _pack_wmat(weight)
    unions = [_build_union(np.asarray(x)[b]) for b in range(B)]
    return [
        _core_inputs(unions[core // DQ], np.asarray(offset), lhsT, core)
        for core in range(N_CORES)
    ]


def assemble_output(results):
    out = np.zeros((B, Cout, D, H, W), np.float32)
    for core in range(N_CORES):
        b, dq = core // DQ, core % DQ
        o = results[core]["out"]
        out[b, :, dq * DO_SLAB:(dq + 1) * DO_SLAB] = o.reshape(
            Cout, DO_SLAB, H, W)
    return out


def kernel(x, offset, weight):
    x = np.asarray(x)
    offset = np.asarray(offset)
    weight = np.asarray(weight)
    nc = _get_nc()
    in_maps = make_in_maps(x, offset, weight)
    res = run_bass_kernel_spmd(nc, in_maps, core_ids=list(range(N_CORES)))
    return assemble_output(res.results)
